# revision 9
# baseline (speedup 1.0000x reference)
"""Binarize kernel for Trainium2, 8-core data-parallel.

out[b, f] = 1.0 if (medians[f] > 0) and (x[b, f] >= medians[f]) else 0.0

Strategy: shard x row-wise across 8 cores (2048 rows each), replicate
medians. On device, build mprime[f] = medians[f] if medians[f] > 0 else 3e38
once (broadcast across 128 partitions), then each [128, 4096] x-tile needs a
single DVE is_ge compare producing 1.0/0.0 directly.

Raw bass (not Tile): walrus codegen only allows one sync-wait command per
compute instruction, so all waits are standalone queue commands. Loads
stream on the SP HWDGE ring, stores on the ACT ring, DVE compute between.
Each buffer slot has its own load/store semaphore: increments on one sem are
serialized by the slot's own dependency chain, so count thresholds are
race-free even though DMA completions across slots can reorder.
"""

import numpy as np

import concourse.bass as bass
import concourse.mybir as mybir
from concourse.bass_utils import run_bass_kernel_spmd

N_CORES = 8
B_FULL = 16384
F = 4096
ROWS = B_FULL // N_CORES  # 2048 rows per core
P = 128
N_TILES = ROWS // P  # 16
NBUF = 4

_BIG = 3.0e38  # pushes the compare threshold above any finite fp32 input


def _build_nc(reps: int = 1) -> bass.Bass:
    """reps > 1 re-runs the whole pipeline on the same data inside one NEFF
    (for slope-based HW timing); the output is identical for any reps."""
    nc = bass.Bass()
    dt = mybir.dt.float32
    x = nc.dram_tensor("x", [ROWS, F], dt, kind="ExternalInput")
    med = nc.dram_tensor("med", [F], dt, kind="ExternalInput")
    out = nc.dram_tensor("out", [ROWS, F], dt, kind="ExternalOutput")

    x_t = x.rearrange("(n p) f -> n p f", p=P)
    o_t = out.rearrange("(n p) f -> n p f", p=P)
    med_b = med[None, :].broadcast_to((P, F))

    import contextlib

    with contextlib.ExitStack() as ctx:
        m_b = ctx.enter_context(nc.sbuf_tensor("m_b", [P, F], dt))
        mprime = ctx.enter_context(nc.sbuf_tensor("mprime", [P, F], dt))
        xt = ctx.enter_context(nc.sbuf_tensor("xt", [P, NBUF, F], dt))
        yt = ctx.enter_context(nc.sbuf_tensor("yt", [P, NBUF, F], dt))
        s_med = ctx.enter_context(nc.semaphore("s_med"))
        s_ld = [
            ctx.enter_context(nc.semaphore(f"s_ld{s}")) for s in range(NBUF)
        ]
        s_st = [
            ctx.enter_context(nc.semaphore(f"s_st{s}")) for s in range(NBUF)
        ]
        s_dve = ctx.enter_context(nc.semaphore("s_dve"))
        block = ctx.enter_context(nc.Block())

        # s_dve counts: +1 per mprime prep op (2 total), then +1 per TT_i,
        # so after TT_i the value is i + 3.

        n_iters = reps * N_TILES

        @block.sync
        def _(sync):
            sync.dma_start(out=m_b[:], in_=med_b).then_inc(s_med, 16)
            for i in range(n_iters):
                s = i % NBUF
                if i >= NBUF:
                    # xt[:, s] still feeding TT_{i-NBUF}
                    sync.wait_ge(s_dve, i - NBUF + 3)
                sync.dma_start(out=xt[:, s], in_=x_t[i % N_TILES]).then_inc(
                    s_ld[s], 16
                )

        @block.scalar
        def _(scalar):
            for i in range(n_iters):
                s = i % NBUF
                scalar.wait_ge(s_dve, i + 3)  # TT_i wrote yt[:, s]
                scalar.dma_start(out=o_t[i % N_TILES], in_=yt[:, s]).then_inc(
                    s_st[s], 16
                )
            # all stores landed before the NEFF retires
            for s in range(NBUF):
                scalar.wait_ge(s_st[s], 16 * (n_iters // NBUF))

        @block.vector
        def _(vector):
            vector.wait_ge(s_med, 16)  # m_b present
            # mprime = (m_b <= 0) * BIG + m_b; sem handshakes order the
            # back-to-back DVE ops (same-engine RAW is not implicit)
            nc.vector.tensor_scalar(
                out=mprime[:],
                in0=m_b[:],
                scalar1=0.0,
                scalar2=_BIG,
                op0=mybir.AluOpType.is_le,
                op1=mybir.AluOpType.mult,
            ).then_inc(s_dve, 1)
            vector.wait_ge(s_dve, 1)
            nc.vector.tensor_add(out=mprime[:], in0=mprime[:], in1=m_b[:]).then_inc(
                s_dve, 1
            )
            vector.wait_ge(s_dve, 2)
            for i in range(n_iters):
                s = i % NBUF
                if i >= NBUF:
                    # yt[:, s] still draining to HBM for tile i-NBUF
                    vector.wait_ge(s_st[s], 16 * (i // NBUF))
                vector.wait_ge(s_ld[s], 16 * (i // NBUF + 1))  # xt[:, s] loaded
                nc.vector.tensor_tensor(
                    out=yt[:, s], in0=xt[:, s], in1=mprime[:], op=mybir.AluOpType.is_ge
                ).then_inc(s_dve, 1)

    return nc


_NC_CACHE: list[bass.Bass] = []


def _get_nc() -> bass.Bass:
    if not _NC_CACHE:
        _NC_CACHE.append(_build_nc())
    return _NC_CACHE[0]


def kernel(x: np.ndarray, medians: np.ndarray) -> np.ndarray:
    x = np.ascontiguousarray(x, dtype=np.float32)
    medians = np.ascontiguousarray(medians, dtype=np.float32)
    assert x.shape == (B_FULL, F), x.shape
    assert medians.shape == (F,), medians.shape

    nc = _get_nc()
    in_maps = [
        {"x": x[c * ROWS : (c + 1) * ROWS], "med": medians} for c in range(N_CORES)
    ]
    res = run_bass_kernel_spmd(nc, in_maps, core_ids=list(range(N_CORES)))
    return np.concatenate([res.results[c]["out"] for c in range(N_CORES)], axis=0)
